# revision 1
# baseline (speedup 1.0000x reference)
"""AdaAttention Trainium2 kernel — data-parallel over batch across 8 NeuronCores.

Full shapes: h [1024,512], sentinel [1024,512], att_feats [1024,96,2048] -> out [1024,512].
Per core: b=128 batch rows. Token axis x = s*128 + b (s-major), N_tok = 12288.

Per-core pipeline (matmuls bf16, psum f32):
  att_feats --SWDGE cast f32->bf16--> nat[b,2048] --xbar transpose--> attf_T[f,x]
  MM1: attT[r,x] = W_aeT.T @ attf_T ; ACT relu(+b_ae) -> bf16
  MM2: att_embdT[a,x] = W_cT.T @ attT ; DVE +h_eT bcast ; ACT tanh(+b_c) -> hAT bf16
  logits row = w_al.T @ hAT -> PE col-transposes -> logits[b,s]
  xbar: attT -> att[x,r] tiles (for cHat)
  sentinel/h prep: PE transposes, sent_eT/h_eT MMs, hA_sent, sentinel logit
  softmax(f32) -> alpha[b,97]; Msel_s = diag(alpha[:,s]) via DVE
  cHat = sum_s Msel_s.T @ att_s (+ sentinel term)  [97 accumulating MMs]
  out = tanh((cHat + h) @ W_oT + b_o) via PE transposes + MM; f32 out.
b_al is skipped everywhere: softmax is invariant to a constant logit shift.
"""
import sys

for p in ("/opt/trn_rl_repo", "/opt/pypackages"):
    if p not in sys.path:
        sys.path.insert(0, p)

import numpy as np
import ml_dtypes
from contextlib import ExitStack

import concourse.bass as bass
import concourse.bacc as bacc
import concourse.mybir as mybir
from concourse import tile

F32 = mybir.dt.float32
BF16 = mybir.dt.bfloat16
AF = mybir.ActivationFunctionType
ALU = mybir.AluOpType

NCORES = 8
B_LOC = 128          # batch rows per core
S = 96               # attention slots
F = 2048             # att feature size
R = 512              # rnn size
A = 512              # att hidden size
NTOK = B_LOC * S     # 12288
XCHUNK = 512         # tokens per pipeline chunk (4 s-tiles)
NCHUNKS = NTOK // XCHUNK       # 24
S_PER_CHUNK = XCHUNK // B_LOC  # 4
FT = F // 128        # 16 f-tiles
RT = R // 128        # 4
AT = A // 128        # 4


def build_nc():
    nc = bacc.Bacc("TRN2", target_bir_lowering=False, debug=False)

    # ---- DRAM parameters (per-core shard shapes) ----
    att_feats = nc.declare_dram_parameter("att_feats", [B_LOC, S, F], F32, isOutput=False)
    h_in = nc.declare_dram_parameter("h", [B_LOC, R], F32, isOutput=False)
    sent_in = nc.declare_dram_parameter("sentinel", [B_LOC, R], F32, isOutput=False)
    # host-prepped weights (bf16, pre-transposed): [p, t, n] = W.T[128*t + p, n]
    w_ae_t = nc.declare_dram_parameter("w_ae_t", [128, FT, R], BF16, isOutput=False)
    w_c_t = nc.declare_dram_parameter("w_c_t", [128, RT, A], BF16, isOutput=False)
    w_s_t = nc.declare_dram_parameter("w_s_t", [128, RT, A], BF16, isOutput=False)
    w_h_t = nc.declare_dram_parameter("w_h_t", [128, RT, A], BF16, isOutput=False)
    w_o_t = nc.declare_dram_parameter("w_o_t", [128, RT, R], BF16, isOutput=False)
    w_al_d = nc.declare_dram_parameter("w_al", [128, AT], BF16, isOutput=False)
    b_ae_d = nc.declare_dram_parameter("b_ae", [128, RT], F32, isOutput=False)
    b_c_d = nc.declare_dram_parameter("b_c", [128, AT], F32, isOutput=False)
    b_s_d = nc.declare_dram_parameter("b_s", [128, AT], F32, isOutput=False)
    b_h_d = nc.declare_dram_parameter("b_h", [128, AT], F32, isOutput=False)
    b_o_d = nc.declare_dram_parameter("b_o", [128, RT], F32, isOutput=False)
    ident_d = nc.declare_dram_parameter("ident", [128, 128], BF16, isOutput=False)
    ident_f32_d = nc.declare_dram_parameter("ident_f32", [128, 128], F32, isOutput=False)
    out_d = nc.declare_dram_parameter("out", [B_LOC, R], F32, isOutput=True)
    attx_dram = nc.dram_tensor("attx_scratch", [NCHUNKS, 128, RT * S_PER_CHUNK * 128], BF16)

    with tile.TileContext(nc) as tc, ExitStack() as ctx:
        # ---- pools ----
        cp = ctx.enter_context(tc.tile_pool(name="consts", bufs=1))
        nat_p = ctx.enter_context(tc.tile_pool(name="nat", bufs=4))
        attf_p = ctx.enter_context(tc.tile_pool(name="attf", bufs=2))
        attT_p = ctx.enter_context(tc.tile_pool(name="attT", bufs=3))
        stg_p = ctx.enter_context(tc.tile_pool(name="stg", bufs=3))
        hat_p = ctx.enter_context(tc.tile_pool(name="hat", bufs=4))
        attx_p = ctx.enter_context(tc.tile_pool(name="attx", bufs=3))
        small_p = ctx.enter_context(tc.tile_pool(name="small", bufs=2))
        soft_p = ctx.enter_context(tc.tile_pool(name="soft", bufs=3))
        msel_p = ctx.enter_context(tc.tile_pool(name="msel", bufs=4))
        ps_mm1 = ctx.enter_context(tc.tile_pool(name="ps_mm1", bufs=2, space="PSUM"))
        ps_mm2 = ctx.enter_context(tc.tile_pool(name="ps_mm2", bufs=2, space="PSUM"))
        ps_small = ctx.enter_context(tc.tile_pool(name="ps_small", bufs=1, space="PSUM"))
        ps_tx = ctx.enter_context(tc.tile_pool(name="ps_tx", bufs=2, space="PSUM"))
        ps_chat = ctx.enter_context(tc.tile_pool(name="ps_chat", bufs=1, space="PSUM"))

        nat_tiles = {}

        def stage_in(c):
            nat = nat_p.tile([B_LOC, S_PER_CHUNK, F], BF16, tag="nat", name=f"nat_{c}")
            for i in range(S_PER_CHUNK):
                s_idx = c * S_PER_CHUNK + i
                if i < 2:
                    nc.gpsimd.dma_start(out=nat[:, i, :], in_=att_feats[:, s_idx, :])
                else:
                    stg = stg_p.tile([B_LOC, F], F32, tag="stg", name=f"stg_{s_idx}")
                    nc.scalar.dma_start(out=stg[:], in_=att_feats[:, s_idx, :])
                    nc.vector.tensor_copy(nat[:, i, :], stg[:])
            nat_tiles[c] = nat

        stage_in(0)
        stage_in(1)

        def const_tile(name, shape, dtype, src):
            t = cp.tile(shape, dtype, tag=name, name=name)
            nc.scalar.dma_start(out=t[:], in_=src[:])
            return t

        # ---- constants / weights ----
        w_ae = const_tile("w_ae", [128, FT, R], BF16, w_ae_t)
        b_ae = const_tile("b_ae", [128, RT], F32, b_ae_d)
        ident = const_tile("ident", [128, 128], BF16, ident_d)
        ident_f32 = const_tile("ident_f32", [128, 128], F32, ident_f32_d)
        w_c = const_tile("w_c", [128, RT, A], BF16, w_c_t)
        w_s = const_tile("w_s", [128, RT, A], BF16, w_s_t)
        w_h = const_tile("w_h", [128, RT, A], BF16, w_h_t)
        w_o = const_tile("w_o", [128, RT, R], BF16, w_o_t)
        wal = const_tile("wal", [128, AT], BF16, w_al_d)
        b_c = const_tile("b_c", [128, AT], F32, b_c_d)
        b_s = const_tile("b_s", [128, AT], F32, b_s_d)
        b_h = const_tile("b_h", [128, AT], F32, b_h_d)
        b_o = const_tile("b_o", [128, RT], F32, b_o_d)

        prep_out = {}

        def prep():
            # ---- h / sentinel prep ----
            h_f32 = const_tile("h_f32", [B_LOC, R], F32, h_in)
            h_bf = cp.tile([B_LOC, R], BF16, tag="h_bf", name="h_bf")
            nc.vector.tensor_copy(h_bf[:], h_f32[:])
            sent_bf = cp.tile([B_LOC, R], BF16, tag="sent_bf", name="sent_bf")
            nc.gpsimd.dma_start(out=sent_bf[:], in_=sent_in[:])  # cast f32->bf16 in DMA

            hT = cp.tile([128, RT, B_LOC], BF16, tag="hT", name="hT")
            sentT = cp.tile([128, RT, B_LOC], BF16, tag="sentT", name="sentT")
            for rb in range(RT):
                pt = ps_small.tile([128, 128], BF16, tag="pssm", name="pt_h")
                nc.tensor.transpose(pt[:], h_bf[:, rb * 128:(rb + 1) * 128], ident[:])
                nc.vector.tensor_copy(hT[:, rb, :], pt[:])
                pt2 = ps_small.tile([128, 128], BF16, tag="pssm", name="pt_s")
                nc.tensor.transpose(pt2[:], sent_bf[:, rb * 128:(rb + 1) * 128], ident[:])
                nc.vector.tensor_copy(sentT[:, rb, :], pt2[:])

            # h_eT[a, b] = (h @ W_hT).T + b_h   [128p(a), AT, 128b] bf16
            h_eT = cp.tile([128, AT, B_LOC], BF16, tag="h_eT", name="h_eT")
            for ab in range(AT):
                psh = ps_small.tile([128, B_LOC], F32, tag="pssm", name="psh")
                for rb in range(RT):
                        nc.tensor.matmul(psh[:], w_h[:, rb, ab * 128:(ab + 1) * 128], hT[:, rb, :],
                                                 start=(rb == 0), stop=(rb == RT - 1))
                nc.scalar.activation(h_eT[:, ab, :], psh[:], AF.Identity,
                                                 bias=b_h[:, ab:ab + 1], scale=1.0)

            # hA_sentT = tanh(sent_eT + h_eT + b_s)   [128p(a), AT, 128b] bf16
            hA_sentT = cp.tile([128, AT, B_LOC], BF16, tag="hA_sentT", name="hA_sentT")
            for ab in range(AT):
                pss = ps_small.tile([128, B_LOC], F32, tag="pssm", name="pss")
                for rb in range(RT):
                        nc.tensor.matmul(pss[:], w_s[:, rb, ab * 128:(ab + 1) * 128], sentT[:, rb, :],
                                                 start=(rb == 0), stop=(rb == RT - 1))
                tmp = small_p.tile([128, B_LOC], F32, tag="preptmp", name="prep_tmp")
                nc.vector.tensor_tensor(out=tmp[:], in0=pss[:], in1=h_eT[:, ab, :], op=ALU.add)
                nc.scalar.activation(hA_sentT[:, ab, :], tmp[:], AF.Tanh,
                                                 bias=b_s[:, ab:ab + 1], scale=1.0)

            # sentinel logit -> logits_sb[:, 0]
            logits_sb = cp.tile([B_LOC, 1 + S], F32, tag="logits", name="logits_sb")
            ps_lr0 = ps_small.tile([1, B_LOC], F32, tag="pssm", name="ps_lr0")
            for ab in range(AT):
                nc.tensor.matmul(ps_lr0[:], wal[:, ab:ab + 1], hA_sentT[:, ab, :],
                                         start=(ab == 0), stop=(ab == AT - 1))
            lrow0 = small_p.tile([1, B_LOC], F32, tag="lrow", name="lrow0")
            nc.vector.tensor_copy(lrow0[:], ps_lr0[:])
            ps_lc0 = ps_small.tile([128, 1], F32, tag="pssm", name="ps_lc0")
            nc.tensor.transpose(ps_lc0[:], lrow0[:], ident_f32[0:1, 0:1])
            nc.vector.tensor_copy(logits_sb[:, 0:1], ps_lc0[:])

            prep_out.update(h_f32=h_f32, h_eT=h_eT, hA_sentT=hA_sentT, logits_sb=logits_sb, sent_bf=sent_bf)

        # ---- main x-chunk pipeline ----
        # stage A(c): 4 SWDGE cast DMAs -> nat chunk tile; 1 xbar -> attf; MM1+relu -> attT chunk tile
        # stage B(c-2): 1 xbar attT -> attx_raw; SWDGE spill; MM2; hA; logits  (deferred 2 chunks so
        #   no engine queue ever holds an op whose deps finish late relative to queue position)
        # attf layout: [128p, 64cc, 128j]: cc = 16*i_s + fc, row f = (cc%16)*128+p, j = b
        # attx layout: [128p, 16cc, 128j]: cc = 4*rb + i_s, x = (cc%4)*128+p, r = (cc//4)*128+j
        attT_chunks = {}

        def stage_mm(c):
            nat = nat_tiles.pop(c)
            attf = attf_p.tile([128, FT * S_PER_CHUNK, 128], BF16, tag="attf", name=f"attf_{c}")
            nc.sync.dma_start(out=attf[:], in_=nat[:], transpose=True)
            # rhs for f-tile fc: [128p(f), (i_s,b)=512] strided AP
            attf4 = attf[:].rearrange("p (i fc) j -> p i fc j", fc=FT)

            attT = attT_p.tile([128, RT, XCHUNK], BF16, tag="attT", name=f"attT_{c}")
            for rb in range(RT):
                ps1 = ps_mm1.tile([128, XCHUNK], F32, tag="mm1", name=f"ps1_{c}_{rb}")
                for fc in range(FT):
                    nc.tensor.matmul(ps1[:], w_ae[:, fc, rb * 128:(rb + 1) * 128],
                                     attf4[:, :, fc, :], start=(fc == 0), stop=(fc == FT - 1))
                nc.scalar.activation(attT[:, rb, :], ps1[:], AF.Relu,
                                     bias=b_ae[:, rb:rb + 1], scale=1.0)
            attT_chunks[c] = attT

        def stage_b(c):
            attT = attT_chunks.pop(c)
            # PE-transpose attT chunk -> attx (same [p, cc=4rb+i, j] layout as xbar would give:
            # block (rb, i): att[x = i*128+p, r = rb*128+j]), spill to DRAM (for cHat)
            axc = attx_p.tile([128, RT * S_PER_CHUNK, 128], BF16, tag="attx", name=f"attx_{c}")
            for rb in range(RT):
                for i in range(S_PER_CHUNK):
                    ptx = ps_tx.tile([128, 128], BF16, tag="ptx", name=f"ptx_{c}_{rb}_{i}")
                    nc.tensor.transpose(ptx[:], attT[:, rb, i * 128:(i + 1) * 128], ident[:])
                    nc.vector.tensor_copy(axc[:, rb * S_PER_CHUNK + i, :], ptx[:])
            nc.gpsimd.dma_start(out=attx_dram[c], in_=axc[:])

            # MM2 -> +h_eT bcast -> tanh(+b_c) -> hAT bf16
            hat_tiles = []
            for ab in range(AT):
                ps2 = ps_mm2.tile([128, XCHUNK], F32, tag="mm2", name=f"ps2_{c}_{ab}")
                for rb in range(RT):
                    nc.tensor.matmul(ps2[:], w_c[:, rb, ab * 128:(ab + 1) * 128],
                                     attT[:, rb, :], start=(rb == 0), stop=(rb == RT - 1))
                tmp = small_p.tile([128, XCHUNK], BF16, tag="hatmp", name=f"hatmp_{c}_{ab}")
                nc.vector.tensor_tensor(
                    out=tmp[:].rearrange("p (s b) -> p s b", s=S_PER_CHUNK),
                    in0=ps2[:].rearrange("p (s b) -> p s b", s=S_PER_CHUNK),
                    in1=h_eT[:, ab, :].unsqueeze(1).broadcast_to([128, S_PER_CHUNK, B_LOC]),
                    op=ALU.add)
                ht = hat_p.tile([128, XCHUNK], BF16, tag="hat", name=f"hat_{c}_{ab}")
                nc.scalar.activation(ht[:], tmp[:], AF.Tanh,
                                     bias=b_c[:, ab:ab + 1], scale=1.0)
                hat_tiles.append(ht)

            # logits row -> col transposes -> logits_sb[:, 1+4c : 1+4c+4]
            ps_l = ps_small.tile([1, XCHUNK], F32, tag="pssm", name=f"ps_l_{c}")
            for ab in range(AT):
                nc.tensor.matmul(ps_l[:], wal[:, ab:ab + 1], hat_tiles[ab][:],
                                 start=(ab == 0), stop=(ab == AT - 1))
            lr = small_p.tile([1, XCHUNK], F32, tag="lrow", name=f"lr_{c}")
            nc.vector.tensor_copy(lr[:], ps_l[:])
            ps_cc = ps_small.tile([128, S_PER_CHUNK], F32, tag="pssm", name=f"ps_cc_{c}")
            for i in range(S_PER_CHUNK):
                nc.tensor.transpose(ps_cc[:, i:i + 1], lr[:, i * 128:(i + 1) * 128],
                                    ident_f32[0:1, 0:1])
            nc.vector.tensor_copy(
                logits_sb[:, 1 + c * S_PER_CHUNK: 1 + (c + 1) * S_PER_CHUNK], ps_cc[:])

        DEFER = 2
        stage_mm(0)
        stage_in(2)
        stage_mm(1)
        stage_in(3)
        prep()
        h_f32 = prep_out["h_f32"]
        h_eT = prep_out["h_eT"]
        hA_sentT = prep_out["hA_sentT"]
        logits_sb = prep_out["logits_sb"]
        sent_bf = prep_out["sent_bf"]
        for c in range(DEFER, NCHUNKS + DEFER):
            if c >= DEFER:
                stage_b(c - DEFER)
            if c < NCHUNKS:
                stage_mm(c)
            if c + 2 < NCHUNKS:
                stage_in(c + 2)

        # ---- softmax over 97 slots (f32) ----
        mx = soft_p.tile([B_LOC, 1], F32, tag="soft", name="mx")
        nc.vector.tensor_reduce(out=mx[:], in_=logits_sb[:], op=ALU.max,
                                axis=mybir.AxisListType.X)
        shifted = soft_p.tile([B_LOC, 1 + S], F32, tag="soft", name="shifted")
        nc.vector.tensor_scalar(out=shifted[:], in0=logits_sb[:], scalar1=mx[:],
                                scalar2=None, op0=ALU.subtract)
        expd = soft_p.tile([B_LOC, 1 + S], F32, tag="soft", name="expd")
        nc.scalar.activation(expd[:], shifted[:], AF.Exp)
        ssum = soft_p.tile([B_LOC, 1], F32, tag="soft", name="ssum")
        nc.vector.tensor_reduce(out=ssum[:], in_=expd[:], op=ALU.add,
                                axis=mybir.AxisListType.X)
        rin = soft_p.tile([B_LOC, 1], F32, tag="soft", name="rin")
        nc.vector.reciprocal(rin[:], ssum[:])
        alpha = cp.tile([B_LOC, 1 + S], F32, tag="alpha", name="alpha")
        nc.vector.tensor_scalar(out=alpha[:], in0=expd[:], scalar1=rin[:],
                                scalar2=None, op0=ALU.mult)

        # ---- cHat: 97 accumulating diag matmuls ----
        ps_cH = ps_chat.tile([B_LOC, R], F32, name="ps_cH")
        ms0 = msel_p.tile([128, 128], BF16, tag="msel", name="ms0")
        nc.vector.tensor_scalar(out=ms0[:], in0=ident[:], scalar1=alpha[:, 0:1],
                                scalar2=None, op0=ALU.mult)
        nc.tensor.matmul(ps_cH[:], ms0[:], sent_bf[:], start=True, stop=False)
        for c in range(NCHUNKS):
            axr = attx_p.tile([128, RT * S_PER_CHUNK, 128], BF16, tag="attx", name=f"attx_rd_{c}")
            nc.gpsimd.dma_start(out=axr[:], in_=attx_dram[c])
            axr4 = axr[:].rearrange("p (rb i) j -> p rb i j", i=S_PER_CHUNK)
            for i in range(S_PER_CHUNK):
                t = c * S_PER_CHUNK + i
                ms = msel_p.tile([128, 128], BF16, tag="msel", name=f"ms_{t}")
                nc.vector.tensor_scalar(out=ms[:], in0=ident[:], scalar1=alpha[:, t + 1:t + 2],
                                        scalar2=None, op0=ALU.mult)
                nc.tensor.matmul(ps_cH[:], ms[:], axr4[:, :, i, :],
                                 start=False, stop=(t == S - 1))

        # ---- final: out = tanh((cHat + h) @ W_oT + b_o) ----
        atten_bf = cp.tile([B_LOC, R], BF16, tag="atten", name="atten_bf")
        nc.vector.tensor_tensor(out=atten_bf[:], in0=ps_cH[:], in1=h_f32[:], op=ALU.add)
        attenT = cp.tile([128, RT, B_LOC], BF16, tag="attenT", name="attenT")
        for rb in range(RT):
            ptf = ps_small.tile([128, 128], BF16, tag="pssm", name=f"ptf_{rb}")
            nc.tensor.transpose(ptf[:], atten_bf[:, rb * 128:(rb + 1) * 128], ident[:])
            nc.vector.tensor_copy(attenT[:, rb, :], ptf[:])
        for ob in range(RT):
            pso = ps_small.tile([128, B_LOC], F32, tag="pssm", name=f"pso_{ob}")
            for rb in range(RT):
                nc.tensor.matmul(pso[:], w_o[:, rb, ob * 128:(ob + 1) * 128], attenT[:, rb, :],
                                 start=(rb == 0), stop=(rb == RT - 1))
            otmp = small_p.tile([128, B_LOC], F32, tag="otmp", name=f"otmp_{ob}")
            nc.scalar.activation(otmp[:], pso[:], AF.Tanh,
                                 bias=b_o[:, ob:ob + 1], scale=1.0)
            ptb = ps_small.tile([128, 128], F32, tag="pssm", name=f"ptb_{ob}")
            nc.tensor.transpose(ptb[:], otmp[:], ident_f32[:])
            ostg = small_p.tile([128, 128], F32, tag="ostg", name=f"ostg_{ob}")
            nc.vector.tensor_copy(ostg[:], ptb[:])
            nc.gpsimd.dma_start(out=out_d[:, ob * 128:(ob + 1) * 128], in_=ostg[:])

    nc.compile()
    return nc


# ---------------- host side ----------------
_NC_CACHE = None


def _get_nc():
    global _NC_CACHE
    if _NC_CACHE is None:
        _NC_CACHE = build_nc()
    return _NC_CACHE


def prep_shared(W_ae, b_ae, W_c, b_c, W_s, b_s, W_h, b_h, W_al, b_al, W_o, b_o):
    bf = ml_dtypes.bfloat16

    def wt(w, nt):  # [p, t, n] = w.T[128*t + p, n]
        wT = np.ascontiguousarray(np.asarray(w, np.float32).T)
        return np.ascontiguousarray(
            wT.reshape(nt, 128, wT.shape[1]).transpose(1, 0, 2)).astype(bf)

    def bt(b, nt):  # [p, t] = b[128*t + p]
        return np.ascontiguousarray(
            np.asarray(b, np.float32).reshape(nt, 128).T).astype(np.float32)

    return {
        "w_ae_t": wt(W_ae, FT),
        "w_c_t": wt(W_c, RT),
        "w_s_t": wt(W_s, RT),
        "w_h_t": wt(W_h, RT),
        "w_o_t": wt(W_o, RT),
        "w_al": np.ascontiguousarray(
            np.asarray(W_al, np.float32)[0].reshape(AT, 128).T).astype(bf),
        "b_ae": bt(b_ae, RT),
        "b_c": bt(b_c, AT),
        "b_s": bt(b_s, AT),
        "b_h": bt(b_h, AT),
        "b_o": bt(b_o, RT),
        "ident": np.eye(128, dtype=bf),
        "ident_f32": np.eye(128, dtype=np.float32),
    }


def make_in_maps(h, sentinel, att_feats, shared):
    h = np.asarray(h, np.float32)
    sentinel = np.asarray(sentinel, np.float32)
    att_feats = np.asarray(att_feats, np.float32)
    in_maps = []
    for i in range(NCORES):
        sl = slice(i * B_LOC, (i + 1) * B_LOC)
        m = dict(shared)
        m["h"] = np.ascontiguousarray(h[sl])
        m["sentinel"] = np.ascontiguousarray(sentinel[sl])
        m["att_feats"] = np.ascontiguousarray(att_feats[sl])
        in_maps.append(m)
    return in_maps


def kernel(h, sentinel, att_feats, W_ae, b_ae, W_c, b_c, W_s, b_s,
           W_h, b_h, W_al, b_al, W_o, b_o):
    shared = prep_shared(W_ae, b_ae, W_c, b_c, W_s, b_s, W_h, b_h, W_al, b_al, W_o, b_o)
    in_maps = make_in_maps(h, sentinel, att_feats, shared)
    nc = _get_nc()
    from concourse.bass_utils import run_bass_kernel_spmd
    res = run_bass_kernel_spmd(nc, in_maps, core_ids=list(range(NCORES)))
    out = np.concatenate([res.results[i]["out"] for i in range(NCORES)], axis=0)
    return np.ascontiguousarray(out.astype(np.float32))


if __name__ == "__main__":
    build_nc()
    print("built ok")



# revision 3
# speedup vs baseline: 1.2310x; 1.2310x over previous
"""AdaAttention Trainium2 kernel — data-parallel over batch across 8 NeuronCores.

Full shapes: h [1024,512], sentinel [1024,512], att_feats [1024,96,2048] -> out [1024,512].
Per core: b=128 batch rows. Token axis x = s*128 + b (s-major), N_tok = 12288.

v2 pipeline (fp8 MM1 + streaming flash-style cHat through W_o):
  out = tanh(cHat @ W_oT + h @ W_oT + b_o), cHat = sum_s alpha_s * img_all_s.
  We accumulate C = sum_s exp(l_s) * (img_all_s @ W_oT) into one persistent PSUM
  bank as chunks stream (no max subtraction: |logits| <~ 12, exp safe in f32),
  then divide by d = sum_s exp(l_s) at the end. No att spill, no PE transposes of att.

Per chunk (4 slots = 512 tokens):
  att_feats --SWDGE cast f32->fp8e4--> nat[b,4,2048]
  4x xbar (u16 view) -> attf[p=g%128, gt, i_s, b]   (g = f//2: fp8 byte pairs)
  MM1 (DoubleRow fp8, w_ae scaled x256): attT[r,x] = relu(psum/256 + b_ae)  bf16
  MM2 bf16 -> +h_eT -> tanh(+b_c) -> hA ; logits row = wal @ hA (PE) -> col transpose
  -> exp (ACT) -> e_sb[:, 1+4c..]
  Z_i[b,o] = attT_i.T @ W_oT  (PE, natural layout) -> bf16
  flash (deferred 1 chunk): C += diag(e_t) @ Z_t   [4 accumulating MMs]
Final: out = tanh(C/d + h@W_oT + b_o). b_al skipped (softmax shift-invariant).
"""
import sys

for p in ("/opt/trn_rl_repo", "/opt/pypackages"):
    if p not in sys.path:
        sys.path.insert(0, p)

import numpy as np
import ml_dtypes
from contextlib import ExitStack

import concourse.bass as bass
import concourse.bacc as bacc
import concourse.mybir as mybir
from concourse import tile

F32 = mybir.dt.float32
BF16 = mybir.dt.bfloat16
FP8 = mybir.dt.float8e4
U16 = mybir.dt.uint16
AF = mybir.ActivationFunctionType
ALU = mybir.AluOpType
DR = mybir.MatmulPerfMode.DoubleRow

NCORES = 8
B_LOC = 128          # batch rows per core
S = 96               # attention slots
F = 2048             # att feature size
R = 512              # rnn size
A = 512              # att hidden size
XCHUNK = 512         # tokens per pipeline chunk (4 s-tiles)
NCHUNKS = (B_LOC * S) // XCHUNK   # 24
S_PER_CHUNK = XCHUNK // B_LOC     # 4
GT = F // 256        # 8 double-row f-tiles (256 f's each)
RT = R // 128        # 4
AT = A // 128        # 4
WSCALE = 256.0       # fp8 weight scale for W_ae


def build_nc():
    nc = bacc.Bacc("TRN2", target_bir_lowering=False, debug=False)

    # ---- DRAM parameters (per-core shard shapes) ----
    att_feats = nc.declare_dram_parameter("att_feats", [B_LOC, S, F], F32, isOutput=False)
    h_in = nc.declare_dram_parameter("h", [B_LOC, R], F32, isOutput=False)
    sent_in = nc.declare_dram_parameter("sentinel", [B_LOC, R], F32, isOutput=False)
    # w_ae_dr[p, gt, i, r] = (W_ae*256).T[f, r], f = 2*(gt*128+p)+i   (fp8)
    w_ae_d = nc.declare_dram_parameter("w_ae_dr", [128, GT, 2, R], FP8, isOutput=False)
    w_c_t = nc.declare_dram_parameter("w_c_t", [128, RT, A], BF16, isOutput=False)
    w_s_t = nc.declare_dram_parameter("w_s_t", [128, RT, A], BF16, isOutput=False)
    w_h_t = nc.declare_dram_parameter("w_h_t", [128, RT, A], BF16, isOutput=False)
    w_o_t = nc.declare_dram_parameter("w_o_t", [128, RT, R], BF16, isOutput=False)
    w_al_d = nc.declare_dram_parameter("w_al", [128, AT], BF16, isOutput=False)
    b_ae_d = nc.declare_dram_parameter("b_ae", [128, RT], F32, isOutput=False)
    b_c_d = nc.declare_dram_parameter("b_c", [128, AT], F32, isOutput=False)
    b_s_d = nc.declare_dram_parameter("b_s", [128, AT], F32, isOutput=False)
    b_h_d = nc.declare_dram_parameter("b_h", [128, AT], F32, isOutput=False)
    b_o_bc_d = nc.declare_dram_parameter("b_o_bcast", [128, R], F32, isOutput=False)
    ident_d = nc.declare_dram_parameter("ident", [128, 128], BF16, isOutput=False)
    ident_f32_d = nc.declare_dram_parameter("ident_f32", [128, 128], F32, isOutput=False)
    out_d = nc.declare_dram_parameter("out", [B_LOC, R], F32, isOutput=True)

    with tile.TileContext(nc) as tc, ExitStack() as ctx:
        # ---- pools ----
        cp = ctx.enter_context(tc.tile_pool(name="consts", bufs=1))
        nat_p = ctx.enter_context(tc.tile_pool(name="nat", bufs=4))
        attf_p = ctx.enter_context(tc.tile_pool(name="attf", bufs=2))
        attT_p = ctx.enter_context(tc.tile_pool(name="attT", bufs=3))
        hat_p = ctx.enter_context(tc.tile_pool(name="hat", bufs=6))
        z_p = ctx.enter_context(tc.tile_pool(name="zt", bufs=3))
        small_p = ctx.enter_context(tc.tile_pool(name="small", bufs=3))
        msel_p = ctx.enter_context(tc.tile_pool(name="msel", bufs=4))
        soft_p = ctx.enter_context(tc.tile_pool(name="soft", bufs=1))
        ps_mm1 = ctx.enter_context(tc.tile_pool(name="ps_mm1", bufs=2, space="PSUM"))
        ps_mm2 = ctx.enter_context(tc.tile_pool(name="ps_mm2", bufs=2, space="PSUM"))
        ps_z = ctx.enter_context(tc.tile_pool(name="ps_z", bufs=2, space="PSUM"))
        ps_chat = ctx.enter_context(tc.tile_pool(name="ps_chat", bufs=1, space="PSUM"))
        ps_small = ctx.enter_context(tc.tile_pool(name="ps_small", bufs=1, space="PSUM"))

        nat_tiles = {}

        def stage_in(c):
            nat = nat_p.tile([B_LOC, S_PER_CHUNK, F], FP8, tag="nat", name=f"nat_{c}")
            for i in range(S_PER_CHUNK):
                s_idx = c * S_PER_CHUNK + i
                nc.gpsimd.dma_start(out=nat[:, i, :], in_=att_feats[:, s_idx, :])
            nat_tiles[c] = nat

        stage_in(0)
        stage_in(1)

        def const_tile(name, shape, dtype, src):
            t = cp.tile(shape, dtype, tag=name, name=name)
            nc.scalar.dma_start(out=t[:], in_=src[:])
            return t

        # ---- constants / weights ----
        w_ae = const_tile("w_ae", [128, GT, 2, R], FP8, w_ae_d)
        b_ae = const_tile("b_ae", [128, RT], F32, b_ae_d)
        ident = const_tile("ident", [128, 128], BF16, ident_d)
        ident_f32 = const_tile("ident_f32", [128, 128], F32, ident_f32_d)
        w_c = const_tile("w_c", [128, RT, A], BF16, w_c_t)
        w_s = const_tile("w_s", [128, RT, A], BF16, w_s_t)
        w_h = const_tile("w_h", [128, RT, A], BF16, w_h_t)
        w_o = const_tile("w_o", [128, RT, R], BF16, w_o_t)
        wal = const_tile("wal", [128, AT], BF16, w_al_d)
        b_c = const_tile("b_c", [128, AT], F32, b_c_d)
        b_s = const_tile("b_s", [128, AT], F32, b_s_d)
        b_h = const_tile("b_h", [128, AT], F32, b_h_d)
        b_o_bc = const_tile("b_o_bc", [128, R], F32, b_o_bc_d)

        # e_sb[:, t] = exp(logit_t), t=0 sentinel, t=1.. att slots
        e_sb = cp.tile([B_LOC, 1 + S], F32, tag="e_sb", name="e_sb")
        prep_out = {}

        def prep():
            # h / sentinel -> bf16 -> transposed [r, b] tiles
            h_bf = cp.tile([B_LOC, R], BF16, tag="h_bf", name="h_bf")
            nc.gpsimd.dma_start(out=h_bf[:], in_=h_in[:])
            sent_bf = cp.tile([B_LOC, R], BF16, tag="sent_bf", name="sent_bf")
            nc.gpsimd.dma_start(out=sent_bf[:], in_=sent_in[:])

            hT = cp.tile([128, RT, B_LOC], BF16, tag="hT", name="hT")
            sentT = cp.tile([128, RT, B_LOC], BF16, tag="sentT", name="sentT")
            for rb in range(RT):
                pt = ps_mm2.tile([128, 1024], BF16, tag="mm2", name=f"pt_h{rb}")
                nc.tensor.transpose(pt[:, :128], h_bf[:, rb * 128:(rb + 1) * 128], ident[:])
                nc.vector.tensor_copy(hT[:, rb, :], pt[:, :128])
                pt2 = ps_mm2.tile([128, 1024], BF16, tag="mm2", name=f"pt_s{rb}")
                nc.tensor.transpose(pt2[:, :128], sent_bf[:, rb * 128:(rb + 1) * 128], ident[:])
                nc.vector.tensor_copy(sentT[:, rb, :], pt2[:, :128])

            # h_eT[a, b] = (h @ W_hT).T + b_h   bf16
            h_eT = cp.tile([128, AT, B_LOC], BF16, tag="h_eT", name="h_eT")
            for ab in range(AT):
                psh = ps_mm2.tile([128, A], F32, tag="mm2", name=f"psh{ab}")
                for rb in range(RT):
                    nc.tensor.matmul(psh[:, :B_LOC], w_h[:, rb, ab * 128:(ab + 1) * 128],
                                     hT[:, rb, :], start=(rb == 0), stop=(rb == RT - 1))
                nc.scalar.activation(h_eT[:, ab, :], psh[:, :B_LOC], AF.Identity,
                                     bias=b_h[:, ab:ab + 1], scale=1.0)

            # hA_sentT = tanh(sent_eT + h_eT + b_s)   bf16
            hA_sentT = cp.tile([128, AT, B_LOC], BF16, tag="hA_sentT", name="hA_sentT")
            for ab in range(AT):
                pss = ps_mm2.tile([128, A], F32, tag="mm2", name=f"pss{ab}")
                for rb in range(RT):
                    nc.tensor.matmul(pss[:, :B_LOC], w_s[:, rb, ab * 128:(ab + 1) * 128],
                                     sentT[:, rb, :], start=(rb == 0), stop=(rb == RT - 1))
                tmp = small_p.tile([128, B_LOC], F32, tag="preptmp", name=f"ptmp{ab}")
                nc.vector.tensor_tensor(out=tmp[:], in0=pss[:, :B_LOC],
                                        in1=h_eT[:, ab, :], op=ALU.add)
                nc.scalar.activation(hA_sentT[:, ab, :], tmp[:], AF.Tanh,
                                     bias=b_s[:, ab:ab + 1], scale=1.0)

            # sentinel logit -> e_sb[:, 0]
            ps_lr0 = ps_small.tile([1, B_LOC], F32, tag="sm", name="ps_lr0")
            for ab in range(AT):
                nc.tensor.matmul(ps_lr0[:], wal[:, ab:ab + 1], hA_sentT[:, ab, :],
                                 start=(ab == 0), stop=(ab == AT - 1))
            lrow0 = small_p.tile([1, B_LOC], F32, tag="lrow", name="lrow0")
            nc.vector.tensor_copy(lrow0[:], ps_lr0[:])
            ps_lc0 = ps_small.tile([128, 1], F32, tag="sm", name="ps_lc0")
            nc.tensor.transpose(ps_lc0[:], lrow0[:], ident_f32[0:1, 0:1])
            nc.scalar.activation(e_sb[:, 0:1], ps_lc0[:], AF.Exp)

            # Zsent[b, o] = sentinel @ W_oT  (bf16), H_o = h @ W_oT + b_o (f32)
            ps_zs = ps_z.tile([128, R], F32, tag="z", name="ps_zs")
            for rb in range(RT):
                nc.tensor.matmul(ps_zs[:], sentT[:, rb, :], w_o[:, rb, :],
                                 start=(rb == 0), stop=(rb == RT - 1))
            zs_sb = cp.tile([B_LOC, R], BF16, tag="zs_sb", name="zs_sb")
            nc.scalar.activation(zs_sb[:], ps_zs[:], AF.Copy)
            ps_ho = ps_z.tile([128, R], F32, tag="z", name="ps_ho")
            for rb in range(RT):
                nc.tensor.matmul(ps_ho[:], hT[:, rb, :], w_o[:, rb, :],
                                 start=(rb == 0), stop=(rb == RT - 1))
            h_o = cp.tile([B_LOC, R], F32, tag="h_o", name="h_o")
            nc.vector.tensor_tensor(out=h_o[:], in0=ps_ho[:], in1=b_o_bc[:], op=ALU.add)

            # open the persistent cHat accumulation with the sentinel term
            ps_cH = ps_chat.tile([B_LOC, R], F32, name="ps_cH")
            ms0 = msel_p.tile([128, 128], BF16, tag="msel", name="ms0")
            nc.vector.tensor_scalar(out=ms0[:], in0=ident[:], scalar1=e_sb[:, 0:1],
                                    scalar2=None, op0=ALU.mult)
            nc.tensor.matmul(ps_cH[:], ms0[:], zs_sb[:], start=True, stop=False,
                             skip_group_check=True)
            prep_out.update(h_eT=h_eT, h_o=h_o, ps_cH=ps_cH)

        # ---- main pipeline stages ----
        attT_chunks = {}
        z_chunks = {}

        def stage_mm1(c):
            nat = nat_tiles.pop(c)
            # 4 per-slot xbar transposes (u16 = fp8 byte pair): attf[p, gt, i_s, b]
            attf = attf_p.tile([128, GT, S_PER_CHUNK, 128], U16, tag="attf", name=f"attf_{c}")
            for i in range(S_PER_CHUNK):
                nc.sync.dma_start(out=attf[:, :, i, :], in_=nat[:, i, :].bitcast(U16),
                                  transpose=True)
            attT = attT_p.tile([128, RT, XCHUNK], BF16, tag="attT", name=f"attT_{c}")
            for rb in range(RT):
                ps1 = ps_mm1.tile([128, XCHUNK], F32, tag="mm1", name=f"ps1_{c}_{rb}")
                for gt in range(GT):
                    rhs = attf[:, gt].bitcast(FP8).rearrange("p s (n two) -> p two s n", two=2)
                    nc.tensor.matmul(ps1[:], w_ae[:, gt, :, rb * 128:(rb + 1) * 128],
                                     rhs, start=(gt == 0), stop=(gt == GT - 1),
                                     perf_mode=DR)
                nc.scalar.activation(attT[:, rb, :], ps1[:], AF.Relu,
                                     bias=b_ae[:, rb:rb + 1], scale=1.0 / WSCALE)
            attT_chunks[c] = attT

        def stage_b1(c):
            attT = attT_chunks.pop(c)
            h_eT = prep_out["h_eT"]
            # MM2 -> +h_eT -> tanh(+b_c) -> hA bf16
            hat_tiles = []
            for ab in range(AT):
                ps2 = ps_mm2.tile([128, XCHUNK], F32, tag="mm2", name=f"ps2_{c}_{ab}")
                for rb in range(RT):
                    nc.tensor.matmul(ps2[:], w_c[:, rb, ab * 128:(ab + 1) * 128],
                                     attT[:, rb, :], start=(rb == 0), stop=(rb == RT - 1))
                tmp = small_p.tile([128, XCHUNK], BF16, tag="hatmp", name=f"hatmp_{c}_{ab}")
                nc.vector.tensor_tensor(
                    out=tmp[:].rearrange("p (s b) -> p s b", s=S_PER_CHUNK),
                    in0=ps2[:].rearrange("p (s b) -> p s b", s=S_PER_CHUNK),
                    in1=h_eT[:, ab, :].unsqueeze(1).broadcast_to([128, S_PER_CHUNK, B_LOC]),
                    op=ALU.add)
                ht = hat_p.tile([128, XCHUNK], BF16, tag="hat", name=f"hat_{c}_{ab}")
                nc.scalar.activation(ht[:], tmp[:], AF.Tanh,
                                     bias=b_c[:, ab:ab + 1], scale=1.0)
                hat_tiles.append(ht)

            # logits row
            ps_l = ps_small.tile([1, XCHUNK], F32, tag="sm", name=f"ps_l_{c}")
            for ab in range(AT):
                nc.tensor.matmul(ps_l[:], wal[:, ab:ab + 1], hat_tiles[ab][:],
                                 start=(ab == 0), stop=(ab == AT - 1))
            lr = small_p.tile([1, XCHUNK], F32, tag="lrow", name=f"lr_{c}")
            nc.vector.tensor_copy(lr[:], ps_l[:])

            # Z_i[b, o] = attT_i.T @ W_oT  (natural layout from PE)
            zt = z_p.tile([128, S_PER_CHUNK, R], BF16, tag="zt", name=f"zt_{c}")
            for i in range(S_PER_CHUNK):
                psz = ps_z.tile([128, R], F32, tag="z", name=f"psz_{c}_{i}")
                for rb in range(RT):
                    nc.tensor.matmul(psz[:], attT[:, rb, i * 128:(i + 1) * 128],
                                     w_o[:, rb, :], start=(rb == 0), stop=(rb == RT - 1))
                if i % 2 == 0:
                    nc.vector.tensor_copy(zt[:, i, :], psz[:])
                else:
                    nc.scalar.activation(zt[:, i, :], psz[:], AF.Copy)
            z_chunks[c] = zt

            # logits row -> col transposes -> exp -> e_sb[:, 1+4c : 5+4c]
            ps_cc = ps_small.tile([128, S_PER_CHUNK], F32, tag="sm", name=f"ps_cc_{c}")
            for i in range(S_PER_CHUNK):
                nc.tensor.transpose(ps_cc[:, i:i + 1], lr[:, i * 128:(i + 1) * 128],
                                    ident_f32[0:1, 0:1])
            nc.scalar.activation(
                e_sb[:, 1 + c * S_PER_CHUNK: 1 + (c + 1) * S_PER_CHUNK], ps_cc[:], AF.Exp)

        def stage_flash(c):
            ps_cH = prep_out["ps_cH"]
            zt = z_chunks.pop(c)
            for i in range(S_PER_CHUNK):
                t = c * S_PER_CHUNK + i
                ms = msel_p.tile([128, 128], BF16, tag="msel", name=f"ms_{t}")
                nc.vector.tensor_scalar(out=ms[:], in0=ident[:], scalar1=e_sb[:, t + 1:t + 2],
                                        scalar2=None, op0=ALU.mult)
                nc.tensor.matmul(ps_cH[:], ms[:], zt[:, i, :],
                                 start=False, stop=(t == S - 1), skip_group_check=True)

        stage_mm1(0)
        stage_in(2)
        stage_mm1(1)
        stage_in(3)
        prep()
        for c in range(2, NCHUNKS + 5):
            if 2 <= c <= NCHUNKS + 1:
                stage_b1(c - 2)
            if c < NCHUNKS:
                stage_mm1(c)
            if 3 <= c <= NCHUNKS + 2:
                stage_flash(c - 3)
            if c + 2 < NCHUNKS:
                stage_in(c + 2)

        # ---- final: out = tanh(C/d + H_o) ----
        ps_cH = prep_out["ps_cH"]
        h_o = prep_out["h_o"]
        dsum = soft_p.tile([B_LOC, 1], F32, tag="soft", name="dsum")
        nc.vector.tensor_reduce(out=dsum[:], in_=e_sb[:], op=ALU.add,
                                axis=mybir.AxisListType.X)
        rin = soft_p.tile([B_LOC, 1], F32, tag="rin", name="rin")
        nc.vector.reciprocal(rin[:], dsum[:])
        chn = soft_p.tile([B_LOC, R], F32, tag="chn", name="chn")
        nc.vector.tensor_scalar(out=chn[:], in0=ps_cH[:], scalar1=rin[:],
                                scalar2=None, op0=ALU.mult)
        pre = soft_p.tile([B_LOC, R], F32, tag="pre", name="pre")
        nc.vector.tensor_tensor(out=pre[:], in0=chn[:], in1=h_o[:], op=ALU.add)
        out_sb = soft_p.tile([B_LOC, R], F32, tag="out_sb", name="out_sb")
        nc.scalar.activation(out_sb[:], pre[:], AF.Tanh)
        nc.sync.dma_start(out=out_d[:], in_=out_sb[:])

    nc.compile()
    return nc


# ---------------- host side ----------------
_NC_CACHE = None


def _get_nc():
    global _NC_CACHE
    if _NC_CACHE is None:
        _NC_CACHE = build_nc()
    return _NC_CACHE


def prep_shared(W_ae, b_ae, W_c, b_c, W_s, b_s, W_h, b_h, W_al, b_al, W_o, b_o):
    bf = ml_dtypes.bfloat16
    f8 = ml_dtypes.float8_e4m3

    def wt(w, nt):  # [p, t, n] = w.T[128*t + p, n]
        wT = np.ascontiguousarray(np.asarray(w, np.float32).T)
        return np.ascontiguousarray(
            wT.reshape(nt, 128, wT.shape[1]).transpose(1, 0, 2)).astype(bf)

    def bt(b, nt):  # [p, t] = b[128*t + p]
        return np.ascontiguousarray(
            np.asarray(b, np.float32).reshape(nt, 128).T).astype(np.float32)

    # w_ae_dr[p, gt, i, r] = (W_ae*WSCALE).T[f, r], f = gt*256 + 2p + i
    waeT = (np.asarray(W_ae, np.float32) * WSCALE).T.astype(f8)  # [F, R]
    w_ae_dr = np.ascontiguousarray(
        waeT.reshape(GT, 128, 2, R).transpose(1, 0, 2, 3))

    return {
        "w_ae_dr": w_ae_dr,
        "w_c_t": wt(W_c, RT),
        "w_s_t": wt(W_s, RT),
        "w_h_t": wt(W_h, RT),
        "w_o_t": wt(W_o, RT),
        "w_al": np.ascontiguousarray(
            np.asarray(W_al, np.float32)[0].reshape(AT, 128).T).astype(bf),
        "b_ae": bt(b_ae, RT),
        "b_c": bt(b_c, AT),
        "b_s": bt(b_s, AT),
        "b_h": bt(b_h, AT),
        "b_o_bcast": np.ascontiguousarray(
            np.tile(np.asarray(b_o, np.float32)[None, :], (128, 1))),
        "ident": np.eye(128, dtype=bf),
        "ident_f32": np.eye(128, dtype=np.float32),
    }


def make_in_maps(h, sentinel, att_feats, shared):
    h = np.asarray(h, np.float32)
    sentinel = np.asarray(sentinel, np.float32)
    att_feats = np.asarray(att_feats, np.float32)
    in_maps = []
    for i in range(NCORES):
        sl = slice(i * B_LOC, (i + 1) * B_LOC)
        m = dict(shared)
        m["h"] = np.ascontiguousarray(h[sl])
        m["sentinel"] = np.ascontiguousarray(sentinel[sl])
        m["att_feats"] = np.ascontiguousarray(att_feats[sl])
        in_maps.append(m)
    return in_maps


def kernel(h, sentinel, att_feats, W_ae, b_ae, W_c, b_c, W_s, b_s,
           W_h, b_h, W_al, b_al, W_o, b_o):
    shared = prep_shared(W_ae, b_ae, W_c, b_c, W_s, b_s, W_h, b_h, W_al, b_al, W_o, b_o)
    in_maps = make_in_maps(h, sentinel, att_feats, shared)
    nc = _get_nc()
    from concourse.bass_utils import run_bass_kernel_spmd
    res = run_bass_kernel_spmd(nc, in_maps, core_ids=list(range(NCORES)))
    out = np.concatenate([res.results[i]["out"] for i in range(NCORES)], axis=0)
    return np.ascontiguousarray(out.astype(np.float32))


if __name__ == "__main__":
    build_nc()
    print("built ok")


# revision 9
# speedup vs baseline: 1.2376x; 1.0053x over previous
"""AdaAttention Trainium2 kernel — data-parallel over batch across 8 NeuronCores.

Full shapes: h [1024,512], sentinel [1024,512], att_feats [1024,96,2048] -> out [1024,512].
Per core: b=128 batch rows; 24 chunks of 4 slots (512 tokens).

v3 pipeline (fp8 DoubleRow MM1, natural-layout MM2, DVE logits, streaming flash cHat):
  out = tanh(cHat @ W_oT + h @ W_oT + b_o), cHat = sum_s alpha_s * img_all_s.
  C = sum_s exp(l_s) * (img_all_s @ W_oT) accumulates in one persistent PSUM bank
  (no max subtraction: |logits| <~ 12, exp safe in f32); divide by d = sum exp at end.

Per chunk:
  att_feats --1 SWDGE DMA, f32->fp8e4 cast--> nat[b,4,2048]
  4x xbar (u16 view) -> attf[p=g%128, gt, i_s, b]  (g = f//2: fp8 byte pairs)
  MM1 (DoubleRow fp8, W_ae x256): attT[r,x] = relu(psum/256 + b_ae)  bf16
  per slot i: MM2' psum[b,a] = attT_i.T @ W_cT (stationary attT blocks)
              hA = tanh(psum + h_e + b_c + b_h)  [DVE add + ACT tanh]
              logit col = DVE tensor_tensor_reduce(hA * wal_rep)      (no PE)
              Z_i[b,o] = attT_i.T @ W_oT -> bf16
  exp (ACT) -> e_sb[:, 1+4c..] ; flash (deferred): C += diag(e_t) @ Z_t
Final: out = tanh(C/d + h@W_oT + b_o). b_al skipped (softmax shift-invariant).
"""
import sys

for p in ("/opt/trn_rl_repo", "/opt/pypackages"):
    if p not in sys.path:
        sys.path.insert(0, p)

import numpy as np
import ml_dtypes
from contextlib import ExitStack

import concourse.bass as bass
import concourse.bacc as bacc
import concourse.mybir as mybir
from concourse import tile

F32 = mybir.dt.float32
BF16 = mybir.dt.bfloat16
FP8 = mybir.dt.float8e4
U16 = mybir.dt.uint16
AF = mybir.ActivationFunctionType
ALU = mybir.AluOpType
DR = mybir.MatmulPerfMode.DoubleRow

NCORES = 8
B_LOC = 128          # batch rows per core
S = 96               # attention slots
F = 2048             # att feature size
R = 512              # rnn size
A = 512              # att hidden size
XCHUNK = 512         # tokens per pipeline chunk (4 s-tiles)
NCHUNKS = (B_LOC * S) // XCHUNK   # 24
S_PER_CHUNK = XCHUNK // B_LOC     # 4
GT = F // 256        # 8 double-row f-tiles (256 f's each)
RT = R // 128        # 4
AT = A // 128        # 4
WSCALE = 256.0       # fp8 weight scale for W_ae


def build_nc():
    nc = bacc.Bacc("TRN2", target_bir_lowering=False, debug=False)

    # ---- DRAM parameters (per-core shard shapes) ----
    att_feats = nc.declare_dram_parameter("att_feats", [B_LOC, S, F], F32, isOutput=False)
    h_in = nc.declare_dram_parameter("h", [B_LOC, R], F32, isOutput=False)
    sent_in = nc.declare_dram_parameter("sentinel", [B_LOC, R], F32, isOutput=False)
    # w_ae_dr[p, gt, i, r] = (W_ae*256).T[f, r], f = 2*(gt*128+p)+i   (fp8)
    w_ae_d = nc.declare_dram_parameter("w_ae_dr", [128, GT, 2, R], FP8, isOutput=False)
    w_c_t = nc.declare_dram_parameter("w_c_t", [128, RT, A], BF16, isOutput=False)
    w_s_t = nc.declare_dram_parameter("w_s_t", [128, RT, A], BF16, isOutput=False)
    w_h_t = nc.declare_dram_parameter("w_h_t", [128, RT, A], BF16, isOutput=False)
    w_o_t = nc.declare_dram_parameter("w_o_t", [128, RT, R], BF16, isOutput=False)
    wal_rep_d = nc.declare_dram_parameter("wal_rep", [128, A], BF16, isOutput=False)
    b_ae_d = nc.declare_dram_parameter("b_ae", [128, RT], F32, isOutput=False)
    bcbh_d = nc.declare_dram_parameter("bcbh", [128, A], F32, isOutput=False)   # b_c + b_h
    bsbh_d = nc.declare_dram_parameter("bsbh", [128, A], F32, isOutput=False)   # b_s + b_h
    b_o_bc_d = nc.declare_dram_parameter("b_o_bcast", [128, R], F32, isOutput=False)
    ident_d = nc.declare_dram_parameter("ident", [128, 128], BF16, isOutput=False)
    out_d = nc.declare_dram_parameter("out", [B_LOC, R], F32, isOutput=True)

    with tile.TileContext(nc) as tc, ExitStack() as ctx:
        # ---- pools ----
        cp = ctx.enter_context(tc.tile_pool(name="consts", bufs=1))
        nat_p = ctx.enter_context(tc.tile_pool(name="nat", bufs=4))
        attf_p = ctx.enter_context(tc.tile_pool(name="attf", bufs=2))
        attT_p = ctx.enter_context(tc.tile_pool(name="attT", bufs=3))
        hat_p = ctx.enter_context(tc.tile_pool(name="hat", bufs=4))
        z_p = ctx.enter_context(tc.tile_pool(name="zt", bufs=3))
        small_p = ctx.enter_context(tc.tile_pool(name="small", bufs=3))
        msel_p = ctx.enter_context(tc.tile_pool(name="msel", bufs=4))
        lcol_p = ctx.enter_context(tc.tile_pool(name="lcol", bufs=3))
        soft_p = ctx.enter_context(tc.tile_pool(name="soft", bufs=1))
        ps_mm1 = ctx.enter_context(tc.tile_pool(name="ps_mm1", bufs=2, space="PSUM"))
        ps_mm2 = ctx.enter_context(tc.tile_pool(name="ps_mm2", bufs=3, space="PSUM"))
        ps_z = ctx.enter_context(tc.tile_pool(name="ps_z", bufs=2, space="PSUM"))
        ps_chat = ctx.enter_context(tc.tile_pool(name="ps_chat", bufs=1, space="PSUM"))

        def const_tile(name, shape, dtype, src):
            t = cp.tile(shape, dtype, tag=name, name=name)
            nc.scalar.dma_start(out=t[:], in_=src[:])
            return t

        # h / sentinel casts first on the gpsimd queue (tiny; unblock prep)
        h_bf = cp.tile([B_LOC, R], BF16, tag="h_bf", name="h_bf")
        nc.gpsimd.dma_start(out=h_bf[:], in_=h_in[:])
        sent_bf = cp.tile([B_LOC, R], BF16, tag="sent_bf", name="sent_bf")
        nc.gpsimd.dma_start(out=sent_bf[:], in_=sent_in[:])

        nat_tiles = {}

        def stage_in(c):
            nat = nat_p.tile([B_LOC, S_PER_CHUNK, F], FP8, tag="nat", name=f"nat_{c}")
            nc.gpsimd.dma_start(
                out=nat[:], in_=att_feats[:, c * S_PER_CHUNK:(c + 1) * S_PER_CHUNK, :])
            nat_tiles[c] = nat

        # ---- constants (prep deps first, then chunk-loop deps) ----
        ident = const_tile("ident", [128, 128], BF16, ident_d)
        w_h = const_tile("w_h", [128, RT, A], BF16, w_h_t)
        w_s = const_tile("w_s", [128, RT, A], BF16, w_s_t)
        w_o = const_tile("w_o", [128, RT, R], BF16, w_o_t)
        wal_rep = const_tile("wal_rep", [128, A], BF16, wal_rep_d)
        bcbh = const_tile("bcbh", [128, A], F32, bcbh_d)
        bsbh = const_tile("bsbh", [128, A], F32, bsbh_d)
        b_o_bc = const_tile("b_o_bc", [128, R], F32, b_o_bc_d)
        w_ae = const_tile("w_ae", [128, GT, 2, R], FP8, w_ae_d)
        b_ae = const_tile("b_ae", [128, RT], F32, b_ae_d)
        w_c = const_tile("w_c", [128, RT, A], BF16, w_c_t)

        stage_in(0)
        stage_in(1)
        stage_in(2)

        # e_sb[:, t] = exp(logit_t), t=0 sentinel, t=1.. att slots
        e_sb = cp.tile([B_LOC, 1 + S], F32, tag="e_sb", name="e_sb")
        prep_out = {}

        def prep():
            hT = cp.tile([128, RT, B_LOC], BF16, tag="hT", name="hT")
            sentT = cp.tile([128, RT, B_LOC], BF16, tag="sentT", name="sentT")
            for rb in range(RT):
                pt = ps_mm2.tile([128, 1024], BF16, tag="mm2", name=f"pt_h{rb}")
                nc.tensor.transpose(pt[:, :128], h_bf[:, rb * 128:(rb + 1) * 128], ident[:])
                nc.vector.tensor_copy(hT[:, rb, :], pt[:, :128])
                pt2 = ps_mm2.tile([128, 1024], BF16, tag="mm2", name=f"pt_s{rb}")
                nc.tensor.transpose(pt2[:, :128], sent_bf[:, rb * 128:(rb + 1) * 128], ident[:])
                nc.vector.tensor_copy(sentT[:, rb, :], pt2[:, :128])

            # h_e (natural [b, a]) -> he_c = h_e + b_c + b_h ; he_s = h_e + b_s + b_h
            ps_he = ps_mm2.tile([128, A], F32, tag="mm2", name="ps_he")
            for rb in range(RT):
                nc.tensor.matmul(ps_he[:], hT[:, rb, :], w_h[:, rb, :],
                                 start=(rb == 0), stop=(rb == RT - 1))
            he_c = cp.tile([B_LOC, A], F32, tag="he_c", name="he_c")
            nc.vector.tensor_tensor(out=he_c[:], in0=ps_he[:], in1=bcbh[:], op=ALU.add)
            he_s = cp.tile([B_LOC, A], F32, tag="he_s", name="he_s")
            nc.vector.tensor_tensor(out=he_s[:], in0=ps_he[:], in1=bsbh[:], op=ALU.add)

            # sentinel: logit_0 and Zsent
            ps_se = ps_mm2.tile([128, A], F32, tag="mm2", name="ps_se")
            for rb in range(RT):
                nc.tensor.matmul(ps_se[:], sentT[:, rb, :], w_s[:, rb, :],
                                 start=(rb == 0), stop=(rb == RT - 1))
            pre0 = small_p.tile([B_LOC, A], BF16, tag="hatmp", name="pre0")
            nc.vector.tensor_tensor(out=pre0[:], in0=ps_se[:], in1=he_s[:], op=ALU.add)
            hA0 = hat_p.tile([B_LOC, A], BF16, tag="hat", name="hA0")
            nc.scalar.activation(hA0[:], pre0[:], AF.Tanh)
            ttr0 = small_p.tile([B_LOC, A], BF16, tag="ttro", name="ttr0")
            lc0 = lcol_p.tile([B_LOC, 1], F32, tag="lc", name="lc0")
            nc.vector.tensor_tensor(out=ttr0[:], in0=hA0[:], in1=wal_rep[:], op=ALU.mult)
            nc.vector.tensor_reduce(out=lc0[:], in_=ttr0[:], op=ALU.add,
                                    axis=mybir.AxisListType.X)
            nc.scalar.activation(e_sb[:, 0:1], lc0[:], AF.Exp)

            ps_zs = ps_z.tile([128, R], F32, tag="z", name="ps_zs")
            for rb in range(RT):
                nc.tensor.matmul(ps_zs[:], sentT[:, rb, :], w_o[:, rb, :],
                                 start=(rb == 0), stop=(rb == RT - 1))
            zs_sb = cp.tile([B_LOC, R], BF16, tag="zs_sb", name="zs_sb")
            nc.scalar.activation(zs_sb[:], ps_zs[:], AF.Copy)

            # H_o = h @ W_oT + b_o (f32)
            ps_ho = ps_z.tile([128, R], F32, tag="z", name="ps_ho")
            for rb in range(RT):
                nc.tensor.matmul(ps_ho[:], hT[:, rb, :], w_o[:, rb, :],
                                 start=(rb == 0), stop=(rb == RT - 1))
            h_o = cp.tile([B_LOC, R], F32, tag="h_o", name="h_o")
            nc.vector.tensor_tensor(out=h_o[:], in0=ps_ho[:], in1=b_o_bc[:], op=ALU.add)

            # open the persistent cHat accumulation with the sentinel term
            ps_cH = ps_chat.tile([B_LOC, R], F32, name="ps_cH")
            ms0 = msel_p.tile([128, 128], BF16, tag="msel", name="ms0")
            nc.vector.tensor_scalar(out=ms0[:], in0=ident[:], scalar1=e_sb[:, 0:1],
                                    scalar2=None, op0=ALU.mult)
            nc.tensor.matmul(ps_cH[:], ms0[:], zs_sb[:], start=True, stop=False,
                             skip_group_check=True)
            prep_out.update(h_o=h_o, ps_cH=ps_cH, he_c=he_c)

        # ---- main pipeline stages ----
        attT_chunks = {}
        z_chunks = {}

        def stage_mm1(c):
            nat = nat_tiles.pop(c)
            # 4 per-slot xbar transposes (u16 = fp8 byte pair): attf[p, gt, i_s, b]
            attf = attf_p.tile([128, GT, S_PER_CHUNK, 128], U16, tag="attf", name=f"attf_{c}")
            for i in range(S_PER_CHUNK):
                nc.sync.dma_start(out=attf[:, :, i, :], in_=nat[:, i, :].bitcast(U16),
                                  transpose=True)
            attT = attT_p.tile([128, RT, XCHUNK], BF16, tag="attT", name=f"attT_{c}")
            for rb in range(RT):
                ps1 = ps_mm1.tile([128, XCHUNK], F32, tag="mm1", name=f"ps1_{c}_{rb}")
                for gt in range(GT):
                    rhs = attf[:, gt].bitcast(FP8).rearrange("p s (n two) -> p two s n", two=2)
                    nc.tensor.matmul(ps1[:], w_ae[:, gt, :, rb * 128:(rb + 1) * 128],
                                     rhs, start=(gt == 0), stop=(gt == GT - 1),
                                     perf_mode=DR)
                nc.scalar.activation(attT[:, rb, :], ps1[:], AF.Relu,
                                     bias=b_ae[:, rb:rb + 1], scale=1.0 / WSCALE)
            attT_chunks[c] = attT

        def stage_b1(c):
            attT = attT_chunks.pop(c)
            he_c = prep_out["he_c"]
            zt = z_p.tile([128, S_PER_CHUNK, R], BF16, tag="zt", name=f"zt_{c}")
            lcol = lcol_p.tile([B_LOC, S_PER_CHUNK], F32, tag="lc", name=f"lcol_{c}")
            for i in range(S_PER_CHUNK):
                # MM2': att_embd natural [b, a], stationary = attT block
                ps2 = ps_mm2.tile([128, A], F32, tag="mm2", name=f"ps2_{c}_{i}")
                for rb in range(RT):
                    nc.tensor.matmul(ps2[:], attT[:, rb, i * 128:(i + 1) * 128],
                                     w_c[:, rb, :], start=(rb == 0), stop=(rb == RT - 1))
                # Z_i = attT_i.T @ W_oT
                psz = ps_z.tile([128, R], F32, tag="z", name=f"psz_{c}_{i}")
                for rb in range(RT):
                    nc.tensor.matmul(psz[:], attT[:, rb, i * 128:(i + 1) * 128],
                                     w_o[:, rb, :], start=(rb == 0), stop=(rb == RT - 1))
                tmp = small_p.tile([B_LOC, A], BF16, tag="hatmp", name=f"hatmp_{c}_{i}")
                nc.vector.tensor_tensor(out=tmp[:], in0=ps2[:], in1=he_c[:], op=ALU.add)
                ht = hat_p.tile([B_LOC, A], BF16, tag="hat", name=f"hat_{c}_{i}")
                nc.scalar.activation(ht[:], tmp[:], AF.Tanh)
                ttro = small_p.tile([B_LOC, A], BF16, tag="ttro", name=f"ttro_{c}_{i}")
                nc.vector.tensor_tensor(out=ttro[:], in0=ht[:], in1=wal_rep[:], op=ALU.mult)
                nc.vector.tensor_reduce(out=lcol[:, i:i + 1], in_=ttro[:], op=ALU.add,
                                        axis=mybir.AxisListType.X)
                if i % 2 == 0:
                    nc.vector.tensor_copy(zt[:, i, :], psz[:])
                else:
                    nc.scalar.activation(zt[:, i, :], psz[:], AF.Copy)
            z_chunks[c] = zt
            nc.scalar.activation(
                e_sb[:, 1 + c * S_PER_CHUNK: 1 + (c + 1) * S_PER_CHUNK], lcol[:], AF.Exp)

        def stage_flash(c):
            ps_cH = prep_out["ps_cH"]
            zt = z_chunks.pop(c)
            for i in range(S_PER_CHUNK):
                t = c * S_PER_CHUNK + i
                ms = msel_p.tile([128, 128], BF16, tag="msel", name=f"ms_{t}")
                nc.vector.tensor_scalar(out=ms[:], in0=ident[:], scalar1=e_sb[:, t + 1:t + 2],
                                        scalar2=None, op0=ALU.mult)
                nc.tensor.matmul(ps_cH[:], ms[:], zt[:, i, :],
                                 start=False, stop=(t == S - 1), skip_group_check=True)

        # build pipeline: prep first (PE busy from ~5us while chunk 0 loads+casts)
        prep()
        stage_mm1(0)
        stage_in(3)
        stage_mm1(1)
        stage_in(4)
        for c in range(2, NCHUNKS + 5):
            if 2 <= c <= NCHUNKS + 1:
                stage_b1(c - 2)
            if c < NCHUNKS:
                stage_mm1(c)
            if 3 <= c <= NCHUNKS + 2:
                stage_flash(c - 3)
            if c + 3 < NCHUNKS:
                stage_in(c + 3)

        # ---- final: out = tanh(C/d + H_o) ----
        ps_cH = prep_out["ps_cH"]
        h_o = prep_out["h_o"]
        dsum = soft_p.tile([B_LOC, 1], F32, tag="soft", name="dsum")
        nc.vector.tensor_reduce(out=dsum[:], in_=e_sb[:], op=ALU.add,
                                axis=mybir.AxisListType.X)
        rin = soft_p.tile([B_LOC, 1], F32, tag="rin", name="rin")
        nc.vector.reciprocal(rin[:], dsum[:])
        chn = soft_p.tile([B_LOC, R], F32, tag="chn", name="chn")
        nc.vector.tensor_scalar(out=chn[:], in0=ps_cH[:], scalar1=rin[:],
                                scalar2=None, op0=ALU.mult)
        pre = soft_p.tile([B_LOC, R], F32, tag="pre", name="pre")
        nc.vector.tensor_tensor(out=pre[:], in0=chn[:], in1=h_o[:], op=ALU.add)
        out_sb = soft_p.tile([B_LOC, R], F32, tag="out_sb", name="out_sb")
        nc.scalar.activation(out_sb[:], pre[:], AF.Tanh)
        nc.sync.dma_start(out=out_d[:], in_=out_sb[:])

    nc.compile()
    return nc


# ---------------- host side ----------------
_NC_CACHE = None


def _get_nc():
    global _NC_CACHE
    if _NC_CACHE is None:
        _NC_CACHE = build_nc()
    return _NC_CACHE


def prep_shared(W_ae, b_ae, W_c, b_c, W_s, b_s, W_h, b_h, W_al, b_al, W_o, b_o):
    bf = ml_dtypes.bfloat16
    f8 = ml_dtypes.float8_e4m3

    def wt(w, nt):  # [p, t, n] = w.T[128*t + p, n]
        wT = np.ascontiguousarray(np.asarray(w, np.float32).T)
        return np.ascontiguousarray(
            wT.reshape(nt, 128, wT.shape[1]).transpose(1, 0, 2)).astype(bf)

    def bt(b, nt):  # [p, t] = b[128*t + p]
        return np.ascontiguousarray(
            np.asarray(b, np.float32).reshape(nt, 128).T).astype(np.float32)

    def rep(v):  # [128, len(v)] f32 replicated rows
        return np.ascontiguousarray(
            np.tile(np.asarray(v, np.float32)[None, :], (128, 1)))

    # w_ae_dr[p, gt, i, r] = (W_ae*WSCALE).T[f, r], f = gt*256 + 2p + i
    waeT = (np.asarray(W_ae, np.float32) * WSCALE).T.astype(f8)  # [F, R]
    w_ae_dr = np.ascontiguousarray(
        waeT.reshape(GT, 128, 2, R).transpose(1, 0, 2, 3))

    return {
        "w_ae_dr": w_ae_dr,
        "w_c_t": wt(W_c, RT),
        "w_s_t": wt(W_s, RT),
        "w_h_t": wt(W_h, RT),
        "w_o_t": wt(W_o, RT),
        "wal_rep": rep(np.asarray(W_al, np.float32)[0]).astype(bf),
        "b_ae": bt(b_ae, RT),
        "bcbh": rep(np.asarray(b_c, np.float32) + np.asarray(b_h, np.float32)),
        "bsbh": rep(np.asarray(b_s, np.float32) + np.asarray(b_h, np.float32)),
        "b_o_bcast": rep(b_o),
        "ident": np.eye(128, dtype=bf),
    }


def make_in_maps(h, sentinel, att_feats, shared):
    h = np.asarray(h, np.float32)
    sentinel = np.asarray(sentinel, np.float32)
    att_feats = np.asarray(att_feats, np.float32)
    in_maps = []
    for i in range(NCORES):
        sl = slice(i * B_LOC, (i + 1) * B_LOC)
        m = dict(shared)
        m["h"] = np.ascontiguousarray(h[sl])
        m["sentinel"] = np.ascontiguousarray(sentinel[sl])
        m["att_feats"] = np.ascontiguousarray(att_feats[sl])
        in_maps.append(m)
    return in_maps


def kernel(h, sentinel, att_feats, W_ae, b_ae, W_c, b_c, W_s, b_s,
           W_h, b_h, W_al, b_al, W_o, b_o):
    shared = prep_shared(W_ae, b_ae, W_c, b_c, W_s, b_s, W_h, b_h, W_al, b_al, W_o, b_o)
    in_maps = make_in_maps(h, sentinel, att_feats, shared)
    nc = _get_nc()
    from concourse.bass_utils import run_bass_kernel_spmd
    res = run_bass_kernel_spmd(nc, in_maps, core_ids=list(range(NCORES)))
    out = np.concatenate([res.results[i]["out"] for i in range(NCORES)], axis=0)
    return np.ascontiguousarray(out.astype(np.float32))


if __name__ == "__main__":
    build_nc()
    print("built ok")


# revision 10
# speedup vs baseline: 1.2669x; 1.0236x over previous
"""AdaAttention Trainium2 kernel — data-parallel over batch across 8 NeuronCores.

Full shapes: h [1024,512], sentinel [1024,512], att_feats [1024,96,2048] -> out [1024,512].
Per core: b=128 batch rows; 24 chunks of 4 slots (512 tokens).

v4 pipeline (fp8 DoubleRow MM1, natural-layout MM2, DVE logits, streaming flash cHat):
  out = tanh(cHat @ W_oT + h @ W_oT + b_o), cHat = sum_s alpha_s * img_all_s.
  C = sum_s exp(l_s) * (img_all_s @ W_oT) accumulates in one persistent PSUM bank
  (no max subtraction: |logits| <~ 12, exp safe in f32); divide by d = sum exp at end.

Per chunk:
  att_feats --1 SWDGE DMA, f32->fp8e4 cast--> nat[b,4,2048]
  4x xbar (u16 view) -> attf[p=g%128, gt, i_s, b]  (g = f//2: fp8 byte pairs)
  MM1 (DoubleRow fp8, W_ae x256): attT[r,x] = relu(psum/256 + b_ae)  bf16
  MM2' (x4 slots): psum[b,a] = attT_i.T @ W_cT (stationary attT blocks)
  hA = tanh(psum + h_e + b_c + b_h) [DVE add + ACT tanh]
  logit col = DVE mult+reduce(hA * wal_rep)   (no PE matmul for logits)
  Z_i[b,o] = attT_i.T @ W_oT -> bf16 ; exp (ACT) -> e_sb[:, 1+4c..]
  flash (deferred 1 chunk): C += diag(e_t) @ Z_t  (diag built on GpSimd)
Final: out = tanh(C/d + h@W_oT + b_o). b_al skipped (softmax shift-invariant).
Startup: chunk-0 cast DMA issued first; prep split so the sentinel cross-engine
chain never head-blocks mm1(0) in the PE FIFO.
"""
import sys

for p in ("/opt/trn_rl_repo", "/opt/pypackages"):
    if p not in sys.path:
        sys.path.insert(0, p)

import numpy as np
import ml_dtypes
from contextlib import ExitStack

import concourse.bass as bass
import concourse.bacc as bacc
import concourse.mybir as mybir
from concourse import tile

F32 = mybir.dt.float32
BF16 = mybir.dt.bfloat16
FP8 = mybir.dt.float8e4
U16 = mybir.dt.uint16
AF = mybir.ActivationFunctionType
ALU = mybir.AluOpType
DR = mybir.MatmulPerfMode.DoubleRow

NCORES = 8
B_LOC = 128          # batch rows per core
S = 96               # attention slots
F = 2048             # att feature size
R = 512              # rnn size
A = 512              # att hidden size
XCHUNK = 512         # tokens per pipeline chunk (4 s-tiles)
NCHUNKS = (B_LOC * S) // XCHUNK   # 24
S_PER_CHUNK = XCHUNK // B_LOC     # 4
GT = F // 256        # 8 double-row f-tiles (256 f's each)
RT = R // 128        # 4
AT = A // 128        # 4
WSCALE = 256.0       # fp8 weight scale for W_ae


def build_nc():
    nc = bacc.Bacc("TRN2", target_bir_lowering=False, debug=False)

    # ---- DRAM parameters (per-core shard shapes) ----
    att_feats = nc.declare_dram_parameter("att_feats", [B_LOC, S, F], F32, isOutput=False)
    h_in = nc.declare_dram_parameter("h", [B_LOC, R], F32, isOutput=False)
    sent_in = nc.declare_dram_parameter("sentinel", [B_LOC, R], F32, isOutput=False)
    # w_ae_dr[p, gt, i, r] = (W_ae*256).T[f, r], f = 2*(gt*128+p)+i   (fp8)
    w_ae_d = nc.declare_dram_parameter("w_ae_dr", [128, GT, 2, R], FP8, isOutput=False)
    w_c_t = nc.declare_dram_parameter("w_c_t", [128, RT, A], BF16, isOutput=False)
    w_s_t = nc.declare_dram_parameter("w_s_t", [128, RT, A], BF16, isOutput=False)
    w_h_t = nc.declare_dram_parameter("w_h_t", [128, RT, A], BF16, isOutput=False)
    w_o_t = nc.declare_dram_parameter("w_o_t", [128, RT, R], BF16, isOutput=False)
    wal_rep_d = nc.declare_dram_parameter("wal_rep", [128, A], BF16, isOutput=False)
    b_ae_d = nc.declare_dram_parameter("b_ae", [128, RT], F32, isOutput=False)
    bcbh_d = nc.declare_dram_parameter("bcbh", [128, A], F32, isOutput=False)   # b_c + b_h
    bsbh_d = nc.declare_dram_parameter("bsbh", [128, A], F32, isOutput=False)   # b_s + b_h
    b_o_bc_d = nc.declare_dram_parameter("b_o_bcast", [128, R], F32, isOutput=False)
    ident_d = nc.declare_dram_parameter("ident", [128, 128], BF16, isOutput=False)
    out_d = nc.declare_dram_parameter("out", [B_LOC, R], F32, isOutput=True)

    with tile.TileContext(nc) as tc, ExitStack() as ctx:
        # ---- pools ----
        cp = ctx.enter_context(tc.tile_pool(name="consts", bufs=1))
        nat_p = ctx.enter_context(tc.tile_pool(name="nat", bufs=4))
        attf_p = ctx.enter_context(tc.tile_pool(name="attf", bufs=2))
        attT_p = ctx.enter_context(tc.tile_pool(name="attT", bufs=3))
        hat_p = ctx.enter_context(tc.tile_pool(name="hat", bufs=6))
        z_p = ctx.enter_context(tc.tile_pool(name="zt", bufs=3))
        small_p = ctx.enter_context(tc.tile_pool(name="small", bufs=6))
        msel_p = ctx.enter_context(tc.tile_pool(name="msel", bufs=4))
        lcol_p = ctx.enter_context(tc.tile_pool(name="lcol", bufs=3))
        soft_p = ctx.enter_context(tc.tile_pool(name="soft", bufs=1))
        ps_mm1 = ctx.enter_context(tc.tile_pool(name="ps_mm1", bufs=2, space="PSUM"))
        ps_mm2 = ctx.enter_context(tc.tile_pool(name="ps_mm2", bufs=3, space="PSUM"))
        ps_z = ctx.enter_context(tc.tile_pool(name="ps_z", bufs=2, space="PSUM"))
        ps_chat = ctx.enter_context(tc.tile_pool(name="ps_chat", bufs=1, space="PSUM"))

        nat_tiles = {}

        def stage_in(c):
            nat = nat_p.tile([B_LOC, S_PER_CHUNK, F], FP8, tag="nat", name=f"nat_{c}")
            nc.gpsimd.dma_start(
                out=nat[:], in_=att_feats[:, c * S_PER_CHUNK:(c + 1) * S_PER_CHUNK, :])
            nat_tiles[c] = nat

        def const_tile(name, shape, dtype, src):
            t = cp.tile(shape, dtype, tag=name, name=name)
            nc.scalar.dma_start(out=t[:], in_=src[:])
            return t

        # chunk 0 feed first; h / sentinel casts next (tiny; unblock prep)
        stage_in(0)
        h_bf = cp.tile([B_LOC, R], BF16, tag="h_bf", name="h_bf")
        nc.gpsimd.dma_start(out=h_bf[:], in_=h_in[:])
        sent_bf = cp.tile([B_LOC, R], BF16, tag="sent_bf", name="sent_bf")
        nc.gpsimd.dma_start(out=sent_bf[:], in_=sent_in[:])

        # ---- constants (mm1 deps first, then prep deps, then stage_b1 deps) ----
        ident = const_tile("ident", [128, 128], BF16, ident_d)
        w_ae = const_tile("w_ae", [128, GT, 2, R], FP8, w_ae_d)
        b_ae = const_tile("b_ae", [128, RT], F32, b_ae_d)
        w_h = const_tile("w_h", [128, RT, A], BF16, w_h_t)
        w_s = const_tile("w_s", [128, RT, A], BF16, w_s_t)
        w_o = const_tile("w_o", [128, RT, R], BF16, w_o_t)
        wal_rep = const_tile("wal_rep", [128, A], BF16, wal_rep_d)
        bcbh = const_tile("bcbh", [128, A], F32, bcbh_d)
        bsbh = const_tile("bsbh", [128, A], F32, bsbh_d)
        b_o_bc = const_tile("b_o_bc", [128, R], F32, b_o_bc_d)
        w_c = const_tile("w_c", [128, RT, A], BF16, w_c_t)

        stage_in(1)

        # e_sb[:, t] = exp(logit_t), t=0 sentinel, t=1.. att slots
        e_sb = cp.tile([B_LOC, 1 + S], F32, tag="e_sb", name="e_sb")
        prep_out = {}

        def prep_a():
            """PE-heavy prep + sentinel logit chain (no PE work after the chain)."""
            hT = cp.tile([128, RT, B_LOC], BF16, tag="hT", name="hT")
            sentT = cp.tile([128, RT, B_LOC], BF16, tag="sentT", name="sentT")
            for rb in range(RT):
                pt = ps_mm2.tile([128, 1024], BF16, tag="mm2", name=f"pt_h{rb}")
                nc.tensor.transpose(pt[:, :128], h_bf[:, rb * 128:(rb + 1) * 128], ident[:])
                nc.vector.tensor_copy(hT[:, rb, :], pt[:, :128])
                pt2 = ps_mm2.tile([128, 1024], BF16, tag="mm2", name=f"pt_s{rb}")
                nc.tensor.transpose(pt2[:, :128], sent_bf[:, rb * 128:(rb + 1) * 128], ident[:])
                nc.vector.tensor_copy(sentT[:, rb, :], pt2[:, :128])

            # h_e (natural [b, a]) -> he_c = h_e + b_c + b_h ; he_s = h_e + b_s + b_h
            ps_he = ps_mm2.tile([128, A], F32, tag="mm2", name="ps_he")
            for rb in range(RT):
                nc.tensor.matmul(ps_he[:], hT[:, rb, :], w_h[:, rb, :],
                                 start=(rb == 0), stop=(rb == RT - 1))
            he_c = cp.tile([B_LOC, A], F32, tag="he_c", name="he_c")
            nc.vector.tensor_tensor(out=he_c[:], in0=ps_he[:], in1=bcbh[:], op=ALU.add)
            he_s = cp.tile([B_LOC, A], F32, tag="he_s", name="he_s")
            nc.vector.tensor_tensor(out=he_s[:], in0=ps_he[:], in1=bsbh[:], op=ALU.add)

            # sentinel embed + Zsent + H_o  (PE)
            ps_se = ps_mm2.tile([128, A], F32, tag="mm2", name="ps_se")
            for rb in range(RT):
                nc.tensor.matmul(ps_se[:], sentT[:, rb, :], w_s[:, rb, :],
                                 start=(rb == 0), stop=(rb == RT - 1))
            ps_zs = ps_z.tile([128, R], F32, tag="z", name="ps_zs")
            for rb in range(RT):
                nc.tensor.matmul(ps_zs[:], sentT[:, rb, :], w_o[:, rb, :],
                                 start=(rb == 0), stop=(rb == RT - 1))
            zs_sb = cp.tile([B_LOC, R], BF16, tag="zs_sb", name="zs_sb")
            nc.scalar.activation(zs_sb[:], ps_zs[:], AF.Copy)
            ps_ho = ps_z.tile([128, R], F32, tag="z", name="ps_ho")
            for rb in range(RT):
                nc.tensor.matmul(ps_ho[:], hT[:, rb, :], w_o[:, rb, :],
                                 start=(rb == 0), stop=(rb == RT - 1))
            h_o = cp.tile([B_LOC, R], F32, tag="h_o", name="h_o")
            nc.vector.tensor_tensor(out=h_o[:], in0=ps_ho[:], in1=b_o_bc[:], op=ALU.add)

            # sentinel logit chain (DVE/ACT only)
            pre0 = small_p.tile([B_LOC, A], BF16, tag="hatmp", name="pre0")
            nc.vector.tensor_tensor(out=pre0[:], in0=ps_se[:], in1=he_s[:], op=ALU.add)
            hA0 = hat_p.tile([B_LOC, A], BF16, tag="hat", name="hA0")
            nc.scalar.activation(hA0[:], pre0[:], AF.Tanh)
            ttr0 = small_p.tile([B_LOC, A], BF16, tag="ttro", name="ttr0")
            lc0 = lcol_p.tile([B_LOC, 1], F32, tag="lc", name="lc0")
            nc.vector.tensor_tensor(out=ttr0[:], in0=hA0[:], in1=wal_rep[:], op=ALU.mult)
            nc.vector.tensor_reduce(out=lc0[:], in_=ttr0[:], op=ALU.add,
                                    axis=mybir.AxisListType.X)
            nc.scalar.activation(e_sb[:, 0:1], lc0[:], AF.Exp)
            prep_out.update(h_o=h_o, he_c=he_c, zs_sb=zs_sb)

        def prep_b():
            """Open the persistent cHat accumulation with the sentinel term."""
            ps_cH = ps_chat.tile([B_LOC, R], F32, name="ps_cH")
            ms0 = msel_p.tile([128, 128], BF16, tag="msel", name="ms0")
            nc.gpsimd.tensor_scalar(out=ms0[:], in0=ident[:], scalar1=e_sb[:, 0:1],
                                    scalar2=None, op0=ALU.mult)
            nc.tensor.matmul(ps_cH[:], ms0[:], prep_out["zs_sb"][:], start=True,
                             stop=False, skip_group_check=True)
            prep_out.update(ps_cH=ps_cH)

        # ---- main pipeline stages ----
        attT_chunks = {}
        z_chunks = {}

        def stage_mm1(c):
            nat = nat_tiles.pop(c)
            # 4 per-slot xbar transposes (u16 = fp8 byte pair): attf[p, gt, i_s, b]
            attf = attf_p.tile([128, GT, S_PER_CHUNK, 128], U16, tag="attf", name=f"attf_{c}")
            for i in range(S_PER_CHUNK):
                nc.sync.dma_start(out=attf[:, :, i, :], in_=nat[:, i, :].bitcast(U16),
                                  transpose=True)
            attT = attT_p.tile([128, RT, XCHUNK], BF16, tag="attT", name=f"attT_{c}")
            for rb in range(RT):
                ps1 = ps_mm1.tile([128, XCHUNK], F32, tag="mm1", name=f"ps1_{c}_{rb}")
                for gt in range(GT):
                    rhs = attf[:, gt].bitcast(FP8).rearrange("p s (n two) -> p two s n", two=2)
                    nc.tensor.matmul(ps1[:], w_ae[:, gt, :, rb * 128:(rb + 1) * 128],
                                     rhs, start=(gt == 0), stop=(gt == GT - 1),
                                     perf_mode=DR)
                nc.scalar.activation(attT[:, rb, :], ps1[:], AF.Relu,
                                     bias=b_ae[:, rb:rb + 1], scale=1.0 / WSCALE)
            attT_chunks[c] = attT

        def stage_b1(c):
            attT = attT_chunks.pop(c)
            he_c = prep_out["he_c"]
            zt = z_p.tile([128, S_PER_CHUNK, R], BF16, tag="zt", name=f"zt_{c}")
            lcol = lcol_p.tile([B_LOC, S_PER_CHUNK], F32, tag="lc", name=f"lcol_{c}")
            # PE: all MM2 groups first
            ps2s = []
            for i in range(S_PER_CHUNK):
                ps2 = ps_mm2.tile([128, A], F32, tag="mm2", name=f"ps2_{c}_{i}")
                for rb in range(RT):
                    nc.tensor.matmul(ps2[:], attT[:, rb, i * 128:(i + 1) * 128],
                                     w_c[:, rb, :], start=(rb == 0), stop=(rb == RT - 1))
                ps2s.append(ps2)
            # DVE adds chase the MM2 groups; ACT tanh chases the adds
            tmps = []
            for i in range(S_PER_CHUNK):
                tmp = small_p.tile([B_LOC, A], BF16, tag="hatmp", name=f"hatmp_{c}_{i}")
                nc.vector.tensor_tensor(out=tmp[:], in0=ps2s[i][:], in1=he_c[:], op=ALU.add)
                tmps.append(tmp)
            hts = []
            for i in range(S_PER_CHUNK):
                ht = hat_p.tile([B_LOC, A], BF16, tag="hat", name=f"hat_{c}_{i}")
                nc.scalar.activation(ht[:], tmps[i][:], AF.Tanh)
                hts.append(ht)
            # PE: Z groups
            pszs = []
            for i in range(S_PER_CHUNK):
                psz = ps_z.tile([128, R], F32, tag="z", name=f"psz_{c}_{i}")
                for rb in range(RT):
                    nc.tensor.matmul(psz[:], attT[:, rb, i * 128:(i + 1) * 128],
                                     w_o[:, rb, :], start=(rb == 0), stop=(rb == RT - 1))
                pszs.append(psz)
            # Z copies split DVE/ACT
            for i in range(S_PER_CHUNK):
                if i % 2 == 0:
                    nc.vector.tensor_copy(zt[:, i, :], pszs[i][:])
                else:
                    nc.scalar.activation(zt[:, i, :], pszs[i][:], AF.Copy)
            # logits: DVE mult + reduce per slot, then one exp (ACT)
            for i in range(S_PER_CHUNK):
                ttro = small_p.tile([B_LOC, A], BF16, tag="ttro", name=f"ttro_{c}_{i}")
                nc.vector.tensor_tensor(out=ttro[:], in0=hts[i][:], in1=wal_rep[:],
                                        op=ALU.mult)
                nc.vector.tensor_reduce(out=lcol[:, i:i + 1], in_=ttro[:], op=ALU.add,
                                        axis=mybir.AxisListType.X)
            z_chunks[c] = zt
            nc.scalar.activation(
                e_sb[:, 1 + c * S_PER_CHUNK: 1 + (c + 1) * S_PER_CHUNK], lcol[:], AF.Exp)

        def stage_flash(c):
            ps_cH = prep_out["ps_cH"]
            zt = z_chunks.pop(c)
            for i in range(S_PER_CHUNK):
                t = c * S_PER_CHUNK + i
                ms = msel_p.tile([128, 128], BF16, tag="msel", name=f"ms_{t}")
                nc.gpsimd.tensor_scalar(out=ms[:], in0=ident[:], scalar1=e_sb[:, t + 1:t + 2],
                                        scalar2=None, op0=ALU.mult)
                nc.tensor.matmul(ps_cH[:], ms[:], zt[:, i, :],
                                 start=False, stop=(t == S - 1), skip_group_check=True)

        # ---- build pipeline ----
        stage_mm1(0)
        prep_a()
        stage_in(2)
        stage_mm1(1)
        stage_in(3)
        prep_b()
        for c in range(2, NCHUNKS + 5):
            if 2 <= c <= NCHUNKS + 1:
                stage_b1(c - 2)
            if c < NCHUNKS:
                stage_mm1(c)
            if 3 <= c <= NCHUNKS + 2:
                stage_flash(c - 3)
            if c + 2 < NCHUNKS:
                stage_in(c + 2)

        # ---- final: out = tanh(C/d + H_o) ----
        ps_cH = prep_out["ps_cH"]
        h_o = prep_out["h_o"]
        dsum = soft_p.tile([B_LOC, 1], F32, tag="soft", name="dsum")
        nc.vector.tensor_reduce(out=dsum[:], in_=e_sb[:], op=ALU.add,
                                axis=mybir.AxisListType.X)
        rin = soft_p.tile([B_LOC, 1], F32, tag="rin", name="rin")
        nc.vector.reciprocal(rin[:], dsum[:])
        chn = soft_p.tile([B_LOC, R], F32, tag="chn", name="chn")
        nc.vector.tensor_scalar(out=chn[:], in0=ps_cH[:], scalar1=rin[:],
                                scalar2=None, op0=ALU.mult)
        pre = soft_p.tile([B_LOC, R], F32, tag="pre", name="pre")
        nc.vector.tensor_tensor(out=pre[:], in0=chn[:], in1=h_o[:], op=ALU.add)
        out_sb = soft_p.tile([B_LOC, R], F32, tag="out_sb", name="out_sb")
        nc.scalar.activation(out_sb[:], pre[:], AF.Tanh)
        nc.sync.dma_start(out=out_d[:], in_=out_sb[:])

    nc.compile()
    return nc


# ---------------- host side ----------------
_NC_CACHE = None


def _get_nc():
    global _NC_CACHE
    if _NC_CACHE is None:
        _NC_CACHE = build_nc()
    return _NC_CACHE


def prep_shared(W_ae, b_ae, W_c, b_c, W_s, b_s, W_h, b_h, W_al, b_al, W_o, b_o):
    bf = ml_dtypes.bfloat16
    f8 = ml_dtypes.float8_e4m3

    def wt(w, nt):  # [p, t, n] = w.T[128*t + p, n]
        wT = np.ascontiguousarray(np.asarray(w, np.float32).T)
        return np.ascontiguousarray(
            wT.reshape(nt, 128, wT.shape[1]).transpose(1, 0, 2)).astype(bf)

    def bt(b, nt):  # [p, t] = b[128*t + p]
        return np.ascontiguousarray(
            np.asarray(b, np.float32).reshape(nt, 128).T).astype(np.float32)

    def rep(v):  # [128, len(v)] f32 replicated rows
        return np.ascontiguousarray(
            np.tile(np.asarray(v, np.float32)[None, :], (128, 1)))

    # w_ae_dr[p, gt, i, r] = (W_ae*WSCALE).T[f, r], f = gt*256 + 2p + i
    waeT = (np.asarray(W_ae, np.float32) * WSCALE).T.astype(f8)  # [F, R]
    w_ae_dr = np.ascontiguousarray(
        waeT.reshape(GT, 128, 2, R).transpose(1, 0, 2, 3))

    return {
        "w_ae_dr": w_ae_dr,
        "w_c_t": wt(W_c, RT),
        "w_s_t": wt(W_s, RT),
        "w_h_t": wt(W_h, RT),
        "w_o_t": wt(W_o, RT),
        "wal_rep": rep(np.asarray(W_al, np.float32)[0]).astype(bf),
        "b_ae": bt(b_ae, RT),
        "bcbh": rep(np.asarray(b_c, np.float32) + np.asarray(b_h, np.float32)),
        "bsbh": rep(np.asarray(b_s, np.float32) + np.asarray(b_h, np.float32)),
        "b_o_bcast": rep(b_o),
        "ident": np.eye(128, dtype=bf),
    }


def make_in_maps(h, sentinel, att_feats, shared):
    h = np.asarray(h, np.float32)
    sentinel = np.asarray(sentinel, np.float32)
    att_feats = np.asarray(att_feats, np.float32)
    in_maps = []
    for i in range(NCORES):
        sl = slice(i * B_LOC, (i + 1) * B_LOC)
        m = dict(shared)
        m["h"] = np.ascontiguousarray(h[sl])
        m["sentinel"] = np.ascontiguousarray(sentinel[sl])
        m["att_feats"] = np.ascontiguousarray(att_feats[sl])
        in_maps.append(m)
    return in_maps


def kernel(h, sentinel, att_feats, W_ae, b_ae, W_c, b_c, W_s, b_s,
           W_h, b_h, W_al, b_al, W_o, b_o):
    shared = prep_shared(W_ae, b_ae, W_c, b_c, W_s, b_s, W_h, b_h, W_al, b_al, W_o, b_o)
    in_maps = make_in_maps(h, sentinel, att_feats, shared)
    nc = _get_nc()
    from concourse.bass_utils import run_bass_kernel_spmd
    res = run_bass_kernel_spmd(nc, in_maps, core_ids=list(range(NCORES)))
    out = np.concatenate([res.results[i]["out"] for i in range(NCORES)], axis=0)
    return np.ascontiguousarray(out.astype(np.float32))


if __name__ == "__main__":
    build_nc()
    print("built ok")
